# revision 10
# baseline (speedup 1.0000x reference)
"""BibdLinear Trainium2 kernel: out = input @ (weight * mask).T

Shapes (hardcoded): input [8192, 4096] f32, weight [4096, 4096] f32,
mask [4096, 4096] f32 -> out [8192, 4096] f32.

Sharding (column-parallel x batch-parallel, 8 cores): 2 batch shards x
4 output-feature shards. Core c handles batch rows [(c//4)*4096, +4096)
and output features [(c%4)*1024, +1024); the host concatenates the 8
output slices.

Per-core device GEMM (Bass/Tile), K=4096 contraction split by dtype:
  - k-tiles 0..17 (K0=18): bf16 operands (x*SX, w*SW planes).
  - k-tiles 18..31 (KQ=14): fp8e4 DoubleRow "hi/lo pair" matmuls:
      stationary pair (x_hi, x_lo*SL), moving pair (w~, w~/SL)
      => psum += x_hi*w~ + x_lo*w~, i.e. x at ~17-bit effective
      precision and only w's single e4m3 quantization (~2.4% rms)
      touching the fp8 fraction. DoubleRow streams at 2x the bf16
      matmul rate, so these k-tiles cost half.
  End-to-end rel err ~1.76e-2 vs the f32 reference (gate 2e-2).
  All planes carry a uniform SX*SW scale, descaled at PSUM eviction
  (DVE tensor_scalar_mul / ACT activation-Copy with scale).

Schedule per core: batch blocks of 256 rows; per block 32 k-tiles x
(2 batch subtiles x 4 feature chunks of N=256) accumulate into 8 PSUM
banks. Each accumulator is evicted immediately after its final matmul
(DVE/ACT alternating, staggered so the next block's matmuls never wait
on a bank); a bf16 k-tile is ordered last to widen the eviction window.
The first SPLIT blocks are special: they run as oc-half phases
(blk0-lo, blk1-lo, ..., blk0-hi, ... on alternating PSUM bank quads),
so the DMA pool only has to deliver HALF the weights while the first
phases compute - this removes the weight-preload startup stall. Weights are
resident in SBUF as per-oc-half k-group tiles; x arrives as per-block
k-group strips with 512B-contiguous descriptors, size-graded (small
first so the first matmul issues ~2us in, large after to respect the
~630ns/DMA HWDGE op rate). Outputs store as bf16 (SWDGE queue), host
upcasts to f32. The last block runs its final 4 k-tiles per-accumulator
(back-to-back per PSUM) so evictions and chunked stores start while
other accumulators still compute, shrinking the drain tail.

TimelineSim: ~365.6us/core (previous fp32r dense baseline: 497.5us).
"""
import numpy as np
import ml_dtypes

import concourse.mybir as mybir
import concourse.tile as tile
from concourse import bacc

# ---------------------------------------------------------------- problem
BATCH, IN_F, OUT_F = 8192, 4096, 4096
B_S, O_S = 2, 4
B, OF = BATCH // B_S, OUT_F // O_S     # 4096, 1024 per core
N_CORES = 8

K0 = 18                                 # bf16 k-tiles
KQ = 32 - K0                            # fp8 DoubleRow k-tiles
KL = K0 * 128
SX, SW, SL = 16.0, 64.0, 32.0

NF = 256                                # matmul moving width
OH = OF // 2                            # oc-half width (512)
F32 = mybir.dt.float32
BF16 = mybir.dt.bfloat16
FP8 = mybir.dt.float8e4
NP_BF16 = ml_dtypes.bfloat16
NP_E4 = ml_dtypes.float8_e4m3

WL_GROUPS = [1, 2, 3, 5, 6, 1]         # bf16 w k-groups (sum K0)
WQ_GROUPS = [4, 10]                    # fp8 w k-groups (sum KQ)
XL_GROUPS0 = [2, 3, 4, 9]              # startup-block bf16 x split
XQ_GROUPS0 = [4, 10]                   # startup-block fp8 x split
SPLIT = 3                              # startup blocks run as oc-half phases
XL_GROUPS = [6, 6, 6]                  # steady bf16 x
XQ_GROUPS = [7, 7]                     # steady fp8 x
WARMUP_MMS = 0                         # junk matmuls to ramp the PE p-state

_NC_CACHE = {}


# ---------------------------------------------------------- device program
def build_nc(iters=1, x_bufs=3, out_bufs=4):
    K = IN_F
    KO = K // 128                      # 32
    OC = OF // NF                      # 4
    NBLK = B // 256                    # 16
    SCL = 1.0 / (SX * SW)

    nc = bacc.Bacc(None, target_bir_lowering=False)

    xl = nc.dram_tensor("xl", [KL, B], BF16, kind="ExternalInput")
    xq = nc.dram_tensor("xq", [KQ * 128, B // 256, 2, 256], FP8,
                        kind="ExternalInput")
    wl = nc.dram_tensor("wl", [KL, OF], BF16, kind="ExternalInput")
    wq = nc.dram_tensor("wq", [KQ * 128, 2, 2, OH], FP8,
                        kind="ExternalInput")
    out = nc.dram_tensor("out", [B, OF], BF16, kind="ExternalOutput")

    xlPK = xl.rearrange("(ko p) b -> p ko b", p=128)
    xqPK = xq.rearrange("(kq p) c t b -> p kq c t b", p=128)
    wlPK = wl.rearrange("(ko p) o -> p ko o", p=128)
    wqPK = wq.rearrange("(kq p) h t o -> p kq h t o", p=128)

    DR = mybir.MatmulPerfMode.DoubleRow

    with tile.TileContext(nc) as tc:
        with (
            tc.tile_pool(name="wpool", bufs=1) as wpool,
            tc.tile_pool(name="xpool", bufs=x_bufs) as xpool,
            tc.tile_pool(name="x0pool", bufs=1) as x0pool,
            tc.tile_pool(name="opool", bufs=out_bufs) as opool,
            tc.tile_pool(name="psum", bufs=1, space="PSUM") as psum_pool,
        ):
            for it in range(iters):
                # w k-tile handles per oc-half: wkl[h][k], wkq[h][kq]
                wkl = [[None] * K0 for _ in range(2)]
                wkq = [[None] * KQ for _ in range(2)]

                def load_wl_group(k0, sz, h):
                    wt = wpool.tile([128, sz, OH], BF16, tag=f"wl{k0}h{h}",
                                    name=f"wl{k0}h{h}_{it}")
                    nc.scalar.dma_start(
                        wt, wlPK[:, k0:k0 + sz, h * OH:(h + 1) * OH])
                    for j in range(sz):
                        wkl[h][k0 + j] = (wt, j)

                def load_wq_group(k0, sz, h):
                    wt = wpool.tile([128, sz, 2, OH], FP8, tag=f"wq{k0}h{h}",
                                    name=f"wq{k0}h{h}_{it}")
                    nc.scalar.dma_start(wt, wqPK[:, k0:k0 + sz, h, :, :])
                    for j in range(sz):
                        wkq[h][k0 + j] = (wt, j)

                def load_w_half(h):
                    for gi, sz in enumerate(WL_GROUPS[:-1]):
                        load_wl_group(sum(WL_GROUPS[:gi]), sz, h)
                    for gi, sz in enumerate(WQ_GROUPS):
                        load_wq_group(sum(WQ_GROUPS[:gi]), sz, h)
                    # group holding the last-visited k-tile (K0-1) goes last
                    load_wl_group(sum(WL_GROUPS[:-1]), WL_GROUPS[-1], h)

                def load_x_groups(blk, groups_l, groups_q, pool, tp):
                    xkl = [None] * K0
                    xkq = [None] * KQ
                    k0 = 0
                    for gi, sz in enumerate(groups_l):
                        xt = pool.tile([128, sz, 256], BF16, tag=f"{tp}l{gi}",
                                       name=f"{tp}l{gi}_{blk}_{it}")
                        nc.sync.dma_start(
                            xt, xlPK[:, k0:k0 + sz,
                                     blk * 256:(blk + 1) * 256])
                        for j in range(sz):
                            xkl[k0 + j] = (xt, j)
                        k0 += sz
                    k0 = 0
                    for gi, sz in enumerate(groups_q):
                        xt = pool.tile([128, sz, 2, 256], FP8,
                                       tag=f"{tp}q{gi}",
                                       name=f"{tp}q{gi}_{blk}_{it}")
                        nc.sync.dma_start(xt, xqPK[:, k0:k0 + sz, blk, :, :])
                        for j in range(sz):
                            xkq[k0 + j] = (xt, j)
                        k0 += sz
                    return xkl, xkq

                def w_rhs(k, oc):
                    h, ocl = divmod(oc, OC // 2)
                    if k < K0:
                        wt, j = wkl[h][k]
                        return wt[:, j, ocl * NF:(ocl + 1) * NF]
                    wt, j = wkq[h][k - K0]
                    return wt[:, j, :, ocl * NF:(ocl + 1) * NF]

                KORDER = (list(range(K0 - 1)) + list(range(K0, KO))
                          + [K0 - 1])

                def gemm(xs, ocs, psget):
                    xkl, xkq = xs
                    for ki, k in enumerate(KORDER):
                        first, lastk = ki == 0, ki == KO - 1
                        for bs in range(2):
                            if k < K0:
                                xt, xj = xkl[k]
                                lhsT = xt[:, xj, bs * 128:(bs + 1) * 128]
                                pm = None
                            else:
                                xt, xj = xkq[k - K0]
                                lhsT = xt[:, xj, :, bs * 128:(bs + 1) * 128]
                                pm = DR
                            for oc in ocs:
                                nc.tensor.matmul(
                                    psget(bs, oc), lhsT, w_rhs(k, oc),
                                    start=first, stop=lastk, perf_mode=pm)

                def evict(ps, bs, oc, ots, use_act):
                    dst = ots[bs][:, oc * NF:(oc + 1) * NF]
                    if use_act:
                        nc.scalar.activation(
                            dst, ps, mybir.ActivationFunctionType.Copy,
                            scale=SCL)
                    else:
                        nc.vector.tensor_scalar_mul(dst, ps, SCL)

                def alloc_ps(base, n, blk):
                    return [psum_pool.tile([128, NF], F32, tag=f"ps{base+i}",
                                           name=f"ps{base+i}_{blk}_{it}")
                            for i in range(n)]

                def store(blk, bs, ots, q):
                    q.dma_start(
                        out[(blk * 2 + bs) * 128:(blk * 2 + bs + 1) * 128,
                            :], ots[bs])

                # ---- PE p-state warmup: junk matmuls with no DMA deps ----
                if WARMUP_MMS and it == 0:
                    wm = x0pool.tile([128, 256], BF16, tag="warm",
                                     name=f"warm_{it}")
                    nc.vector.memset(wm[:], 0.0)
                    wps = psum_pool.tile([128, NF], F32, tag="ps7",
                                         name=f"warmps_{it}")
                    for i in range(WARMUP_MMS):
                        nc.tensor.matmul(wps, wm[:, 0:128], wm,
                                         start=(i == 0),
                                         stop=(i == WARMUP_MMS - 1))

                # ---- startup: blocks 0..SPLIT-1 as oc-half phases --------
                xs_start = [
                    load_x_groups(blk, XL_GROUPS0, XQ_GROUPS0, x0pool,
                                  f"x{'abcd'[blk]}")
                    for blk in range(SPLIT)
                ]
                load_w_half(0)
                load_w_half(1)
                # steady-state strips for the next two blocks
                xs_pre = {blk: load_x_groups(blk, XL_GROUPS, XQ_GROUPS,
                                             xpool, "x")
                          for blk in range(SPLIT, SPLIT + 2)}

                ots_start = [
                    [opool.tile([128, OF], BF16, tag=f"ot{bs}",
                                name=f"ot{bs}_{blk}_{it}")
                     for bs in range(2)]
                    for blk in range(SPLIT)
                ]
                for ph in range(2 * SPLIT):
                    blk, h = ph % SPLIT, ph // SPLIT
                    ocs = [2 * h, 2 * h + 1]
                    base = 4 * (ph % 2)
                    ps = alloc_ps(base, 4, blk * 10 + h)
                    gemm(xs_start[blk], ocs, lambda bs, oc, ps=ps, h=h:
                         ps[bs * 2 + (oc - 2 * h)])
                    for i in range(4):
                        bs, ocl = divmod(i, 2)
                        evict(ps[i], bs, 2 * h + ocl, ots_start[blk], i % 2)
                    if h == 1:
                        for bs in range(2):
                            store(blk, bs, ots_start[blk], nc.gpsimd)

                # ---- steady blocks ---------------------------------------
                for blk in range(SPLIT, NBLK):
                    xs = xs_pre.pop(blk)
                    if blk + 2 < NBLK:
                        xs_pre[blk + 2] = load_x_groups(
                            blk + 2, XL_GROUPS, XQ_GROUPS, xpool, "x")
                    psums = alloc_ps(0, 8, blk)
                    ots = [opool.tile([128, OF], BF16, tag=f"ot{bs}",
                                      name=f"ot{bs}_{blk}_{it}")
                           for bs in range(2)]
                    last = blk == NBLK - 1
                    if not last:
                        gemm(xs, range(OC),
                             lambda bs, oc: psums[bs * OC + oc])
                        for i in range(8):
                            bs, oc = divmod(i, OC)
                            evict(psums[i], bs, oc, ots, i % 2)
                        for bs in range(2):
                            store(blk, bs, ots, nc.gpsimd)
                    else:
                        # per-psum staggered tail: each accumulator's final
                        # k-tiles run back-to-back so its eviction and store
                        # chunk start while other accumulators still compute
                        NTAIL = 4
                        xkl, xkq = xs
                        for ki, k in enumerate(KORDER[:-NTAIL]):
                            first = ki == 0
                            for bs in range(2):
                                if k < K0:
                                    xt, xj = xkl[k]
                                    lhsT = xt[:, xj,
                                              bs * 128:(bs + 1) * 128]
                                    pm = None
                                else:
                                    xt, xj = xkq[k - K0]
                                    lhsT = xt[:, xj, :,
                                              bs * 128:(bs + 1) * 128]
                                    pm = DR
                                for oc in range(OC):
                                    nc.tensor.matmul(
                                        psums[bs * OC + oc], lhsT,
                                        w_rhs(k, oc),
                                        start=first, stop=False,
                                        perf_mode=pm)
                        tail_ks = KORDER[-NTAIL:]
                        chunk_q = [nc.sync, nc.gpsimd, nc.sync, nc.gpsimd]
                        for i in range(8):
                            bs, oc = divmod(i, OC)
                            for k in tail_ks:
                                if k < K0:
                                    xt, xj = xkl[k]
                                    lhsT = xt[:, xj,
                                              bs * 128:(bs + 1) * 128]
                                    pm = None
                                else:
                                    xt, xj = xkq[k - K0]
                                    lhsT = xt[:, xj, :,
                                              bs * 128:(bs + 1) * 128]
                                    pm = DR
                                nc.tensor.matmul(
                                    psums[i], lhsT, w_rhs(k, oc),
                                    start=False, stop=(k == tail_ks[-1]),
                                    perf_mode=pm)
                            evict(psums[i], bs, oc, ots, i % 2)
                            if i % 2 == 1:
                                row = (blk * 2 + bs) * 128
                                c0 = (oc - 1) * NF
                                chunk_q[(i - 1) // 2].dma_start(
                                    out[row:row + 128, c0:c0 + 2 * NF],
                                    ots[bs][:, c0:c0 + 2 * NF])

    nc.compile()
    return nc


def _get_nc():
    if "nc" not in _NC_CACHE:
        _NC_CACHE["nc"] = build_nc()
    return _NC_CACHE["nc"]


# ------------------------------------------------------------- host prep
def _prep_x(xs):
    """xs [B, 4096] f32 (batch shard) -> xl bf16, xq packed fp8 pairs."""
    xt = np.ascontiguousarray(xs.T) * SX           # [4096, B] scaled
    xl = xt[:KL].astype(NP_BF16)
    q = xt[KL:]
    hi = q.astype(NP_E4)
    lo = ((q - hi.astype(np.float32)) * SL).astype(NP_E4)
    pair = np.stack([hi, lo], axis=1)              # [KQ*128, 2, B]
    xq = np.ascontiguousarray(
        pair.reshape(KQ * 128, 2, B // 256, 256).transpose(0, 2, 1, 3))
    return xl, xq


def _prep_w(ws):
    """ws [OF, 4096] f32 (masked weight shard) -> wl bf16, wq fp8 pairs."""
    wt = np.ascontiguousarray(ws.T) * SW           # [4096, OF] scaled
    wl = wt[:KL].astype(NP_BF16)
    q = wt[KL:]
    hi = q.astype(NP_E4)
    lo = (q / SL).astype(NP_E4)
    # [KQ*128, 2(oc-half), 2(hi/lo), OH] so per-half DMAs are contiguous
    wq = np.ascontiguousarray(
        np.stack([hi.reshape(-1, 2, OH), lo.reshape(-1, 2, OH)], axis=2))
    return wl, wq


def shard_inputs(input, weight, mask):
    x = np.asarray(input, dtype=np.float32)
    s = np.asarray(weight, dtype=np.float32) * np.asarray(mask,
                                                          dtype=np.float32)
    xparts = [_prep_x(x[i * B:(i + 1) * B]) for i in range(B_S)]
    wparts = [_prep_w(s[j * OF:(j + 1) * OF]) for j in range(O_S)]
    in_maps = []
    for c in range(N_CORES):
        xl, xq = xparts[c // O_S]
        wl, wq = wparts[c % O_S]
        in_maps.append({"xl": xl, "xq": xq, "wl": wl, "wq": wq})
    return in_maps


def gather_output(results):
    outp = np.empty((BATCH, OUT_F), np.float32)
    for c in range(N_CORES):
        b0 = (c // O_S) * B
        o0 = (c % O_S) * OF
        outp[b0:b0 + B, o0:o0 + OF] = results[c]["out"].astype(np.float32)
    return outp


def kernel(input, weight, mask):
    from concourse.bass_utils import run_bass_kernel_spmd
    in_maps = shard_inputs(input, weight, mask)
    res = run_bass_kernel_spmd(_get_nc(), in_maps,
                               core_ids=list(range(N_CORES)))
    return gather_output(res.results)


# revision 12
# speedup vs baseline: 1429.9341x; 1429.9341x over previous
"""BibdLinear Trainium2 kernel: out = input @ (weight * mask).T

Shapes (hardcoded): input [8192, 4096] f32, weight [4096, 4096] f32,
mask [4096, 4096] f32 -> out [8192, 4096] f32.

Sharding (column-parallel x batch-parallel, 8 cores): 2 batch shards x
4 output-feature shards. Core c handles batch rows [(c//4)*4096, +4096)
and output features [(c%4)*1024, +1024); the host concatenates the 8
output slices.

Per-core device GEMM (Bass/Tile), K=4096 contraction split by dtype:
  - k-tiles 0..17 (K0=18): bf16 operands (x*SX, w*SW planes).
  - k-tiles 18..31 (KQ=14): fp8e4 DoubleRow "hi/lo pair" matmuls:
      stationary pair (x_hi, x_lo*SL), moving pair (w~, w~/SL)
      => psum += x_hi*w~ + x_lo*w~, i.e. x at ~17-bit effective
      precision and only w's single e4m3 quantization (~2.4% rms)
      touching the fp8 fraction. DoubleRow streams at 2x the bf16
      matmul rate, so these k-tiles cost half.
  End-to-end rel err ~1.76e-2 vs the f32 reference (gate 2e-2).
  All planes carry a uniform SX*SW scale, descaled at PSUM eviction
  (DVE tensor_scalar_mul / ACT activation-Copy with scale).

Schedule per core: batch blocks of 256 rows; per block 32 k-tiles x
(2 batch subtiles x 4 feature chunks of N=256) accumulate into 8 PSUM
banks. Each accumulator is evicted immediately after its final matmul
(DVE/ACT alternating, staggered so the next block's matmuls never wait
on a bank); a bf16 k-tile is ordered last to widen the eviction window.
Blocks 0-1 are special: they run k-interleaved as two oc-half phases
(both blocks' lo halves, then both hi halves), so the DMA pool only
has to deliver half the weights, spread across a full 21us phase,
while the first blocks compute - removing the weight-preload startup
stall. Weights are
resident in SBUF as per-oc-half k-group tiles; x arrives as per-block
k-group strips with 512B-contiguous descriptors, size-graded (small
first so the first matmul issues ~2us in, large after to respect the
~630ns/DMA HWDGE op rate). Outputs store as bf16 (SWDGE queue), host
upcasts to f32. The last block runs its final 4 k-tiles per-accumulator
(back-to-back per PSUM) so evictions and chunked stores start while
other accumulators still compute, shrinking the drain tail.

TimelineSim: ~365.6us/core (previous fp32r dense baseline: 497.5us).
"""
import numpy as np
import ml_dtypes

import concourse.mybir as mybir
import concourse.tile as tile
from concourse import bacc

# ---------------------------------------------------------------- problem
BATCH, IN_F, OUT_F = 8192, 4096, 4096
B_S, O_S = 2, 4
B, OF = BATCH // B_S, OUT_F // O_S     # 4096, 1024 per core
N_CORES = 8

K0 = 18                                 # bf16 k-tiles
KQ = 32 - K0                            # fp8 DoubleRow k-tiles
KL = K0 * 128
SX, SW, SL = 16.0, 64.0, 32.0

NF = 256                                # matmul moving width
OH = OF // 2                            # oc-half width (512)
F32 = mybir.dt.float32
BF16 = mybir.dt.bfloat16
FP8 = mybir.dt.float8e4
NP_BF16 = ml_dtypes.bfloat16
NP_E4 = ml_dtypes.float8_e4m3

WL_GROUPS = [1, 2, 3, 5, 6, 1]         # bf16 w k-groups (sum K0)
WQ_GROUPS = [4, 10]                    # fp8 w k-groups (sum KQ)
XL_GROUPS0 = [2, 3, 4, 9]              # startup-block bf16 x split
XQ_GROUPS0 = [4, 10]                   # startup-block fp8 x split
XL_GROUPS = [6, 6, 6]                  # steady bf16 x
XQ_GROUPS = [7, 7]                     # steady fp8 x
WARMUP_MMS = 0                         # junk matmuls to ramp the PE p-state

_NC_CACHE = {}


# ---------------------------------------------------------- device program
def build_nc(iters=1, x_bufs=3, out_bufs=4):
    K = IN_F
    KO = K // 128                      # 32
    OC = OF // NF                      # 4
    NBLK = B // 256                    # 16
    SCL = 1.0 / (SX * SW)

    nc = bacc.Bacc(None, target_bir_lowering=False)

    xl = nc.dram_tensor("xl", [KL, B], BF16, kind="ExternalInput")
    xq = nc.dram_tensor("xq", [KQ * 128, B // 256, 2, 256], FP8,
                        kind="ExternalInput")
    wl = nc.dram_tensor("wl", [KL, OF], BF16, kind="ExternalInput")
    wq = nc.dram_tensor("wq", [KQ * 128, 2, 2, OH], FP8,
                        kind="ExternalInput")
    out = nc.dram_tensor("out", [B, OF], BF16, kind="ExternalOutput")

    xlPK = xl.rearrange("(ko p) b -> p ko b", p=128)
    xqPK = xq.rearrange("(kq p) c t b -> p kq c t b", p=128)
    wlPK = wl.rearrange("(ko p) o -> p ko o", p=128)
    wqPK = wq.rearrange("(kq p) h t o -> p kq h t o", p=128)

    DR = mybir.MatmulPerfMode.DoubleRow

    with tile.TileContext(nc) as tc:
        with (
            tc.tile_pool(name="wpool", bufs=1) as wpool,
            tc.tile_pool(name="xpool", bufs=x_bufs) as xpool,
            tc.tile_pool(name="x0pool", bufs=1) as x0pool,
            tc.tile_pool(name="opool", bufs=out_bufs) as opool,
            tc.tile_pool(name="psum", bufs=1, space="PSUM") as psum_pool,
        ):
            for it in range(iters):
                # w k-tile handles per oc-half: wkl[h][k], wkq[h][kq]
                wkl = [[None] * K0 for _ in range(2)]
                wkq = [[None] * KQ for _ in range(2)]

                def load_wl_group(k0, sz, h):
                    wt = wpool.tile([128, sz, OH], BF16, tag=f"wl{k0}h{h}",
                                    name=f"wl{k0}h{h}_{it}")
                    nc.scalar.dma_start(
                        wt, wlPK[:, k0:k0 + sz, h * OH:(h + 1) * OH])
                    for j in range(sz):
                        wkl[h][k0 + j] = (wt, j)

                def load_wq_group(k0, sz, h):
                    wt = wpool.tile([128, sz, 2, OH], FP8, tag=f"wq{k0}h{h}",
                                    name=f"wq{k0}h{h}_{it}")
                    nc.scalar.dma_start(wt, wqPK[:, k0:k0 + sz, h, :, :])
                    for j in range(sz):
                        wkq[h][k0 + j] = (wt, j)

                def load_w_half(h):
                    for gi, sz in enumerate(WL_GROUPS[:-1]):
                        load_wl_group(sum(WL_GROUPS[:gi]), sz, h)
                    for gi, sz in enumerate(WQ_GROUPS):
                        load_wq_group(sum(WQ_GROUPS[:gi]), sz, h)
                    # group holding the last-visited k-tile (K0-1) goes last
                    load_wl_group(sum(WL_GROUPS[:-1]), WL_GROUPS[-1], h)

                def load_x_groups(blk, groups_l, groups_q, pool, tp):
                    xkl = [None] * K0
                    xkq = [None] * KQ
                    k0 = 0
                    for gi, sz in enumerate(groups_l):
                        xt = pool.tile([128, sz, 256], BF16, tag=f"{tp}l{gi}",
                                       name=f"{tp}l{gi}_{blk}_{it}")
                        nc.sync.dma_start(
                            xt, xlPK[:, k0:k0 + sz,
                                     blk * 256:(blk + 1) * 256])
                        for j in range(sz):
                            xkl[k0 + j] = (xt, j)
                        k0 += sz
                    k0 = 0
                    for gi, sz in enumerate(groups_q):
                        xt = pool.tile([128, sz, 2, 256], FP8,
                                       tag=f"{tp}q{gi}",
                                       name=f"{tp}q{gi}_{blk}_{it}")
                        nc.sync.dma_start(xt, xqPK[:, k0:k0 + sz, blk, :, :])
                        for j in range(sz):
                            xkq[k0 + j] = (xt, j)
                        k0 += sz
                    return xkl, xkq

                def w_rhs(k, oc):
                    h, ocl = divmod(oc, OC // 2)
                    if k < K0:
                        wt, j = wkl[h][k]
                        return wt[:, j, ocl * NF:(ocl + 1) * NF]
                    wt, j = wkq[h][k - K0]
                    return wt[:, j, :, ocl * NF:(ocl + 1) * NF]

                KORDER = (list(range(K0 - 1)) + list(range(K0, KO))
                          + [K0 - 1])

                def gemm(xs, ocs, psget):
                    xkl, xkq = xs
                    for ki, k in enumerate(KORDER):
                        first, lastk = ki == 0, ki == KO - 1
                        for bs in range(2):
                            if k < K0:
                                xt, xj = xkl[k]
                                lhsT = xt[:, xj, bs * 128:(bs + 1) * 128]
                                pm = None
                            else:
                                xt, xj = xkq[k - K0]
                                lhsT = xt[:, xj, :, bs * 128:(bs + 1) * 128]
                                pm = DR
                            for oc in ocs:
                                nc.tensor.matmul(
                                    psget(bs, oc), lhsT, w_rhs(k, oc),
                                    start=first, stop=lastk, perf_mode=pm)

                def evict(ps, bs, oc, ots, use_act):
                    dst = ots[bs][:, oc * NF:(oc + 1) * NF]
                    if use_act:
                        nc.scalar.activation(
                            dst, ps, mybir.ActivationFunctionType.Copy,
                            scale=SCL)
                    else:
                        nc.vector.tensor_scalar_mul(dst, ps, SCL)

                def alloc_ps(base, n, blk):
                    return [psum_pool.tile([128, NF], F32, tag=f"ps{base+i}",
                                           name=f"ps{base+i}_{blk}_{it}")
                            for i in range(n)]

                def store(blk, bs, ots, q):
                    q.dma_start(
                        out[(blk * 2 + bs) * 128:(blk * 2 + bs + 1) * 128,
                            :], ots[bs])

                # ---- PE p-state warmup: junk matmuls with no DMA deps ----
                if WARMUP_MMS and it == 0:
                    wm = x0pool.tile([128, 256], BF16, tag="warm",
                                     name=f"warm_{it}")
                    nc.vector.memset(wm[:], 0.0)
                    wps = psum_pool.tile([128, NF], F32, tag="ps7",
                                         name=f"warmps_{it}")
                    for i in range(WARMUP_MMS):
                        nc.tensor.matmul(wps, wm[:, 0:128], wm,
                                         start=(i == 0),
                                         stop=(i == WARMUP_MMS - 1))

                # ---- startup: blocks 0,1 interleaved, one oc-half per
                # phase: halves the early weight demand AND spreads each
                # w-half over a full 2-block phase (21.4us) ------------
                xs_start = [
                    load_x_groups(blk, XL_GROUPS0, XQ_GROUPS0, x0pool,
                                  f"x{'ab'[blk]}")
                    for blk in range(2)
                ]
                load_w_half(0)
                load_w_half(1)
                xs_pre = {blk: load_x_groups(blk, XL_GROUPS, XQ_GROUPS,
                                             xpool, "x")
                          for blk in (2, 3)}

                ots_start = [
                    [opool.tile([128, OF], BF16, tag=f"ot{bs}",
                                name=f"ot{bs}_{blk}_{it}")
                     for bs in range(2)]
                    for blk in range(2)
                ]
                EVORD = [0, 1, 4, 5, 2, 3, 6, 7]   # psum completion order
                for h in (0, 1):
                    ocs = [2 * h, 2 * h + 1]
                    ps = alloc_ps(0, 8, 100 + h)
                    for ki, k in enumerate(KORDER):
                        first, lastk = ki == 0, ki == KO - 1
                        for bs in range(2):
                            for blki in range(2):
                                xkl, xkq = xs_start[blki]
                                if k < K0:
                                    xt, xj = xkl[k]
                                    lhsT = xt[:, xj,
                                              bs * 128:(bs + 1) * 128]
                                    pm = None
                                else:
                                    xt, xj = xkq[k - K0]
                                    lhsT = xt[:, xj, :,
                                              bs * 128:(bs + 1) * 128]
                                    pm = DR
                                for oc in ocs:
                                    nc.tensor.matmul(
                                        ps[blki * 4 + bs * 2 + (oc - 2 * h)],
                                        lhsT, w_rhs(k, oc),
                                        start=first, stop=lastk,
                                        perf_mode=pm)
                    for n, i in enumerate(EVORD):
                        blki, r = divmod(i, 4)
                        bs, ocl = divmod(r, 2)
                        evict(ps[i], bs, 2 * h + ocl, ots_start[blki], n % 2)
                    if h == 1:
                        for blk in range(2):
                            for bs in range(2):
                                store(blk, bs, ots_start[blk], nc.gpsimd)

                # ---- steady blocks ---------------------------------------
                for blk in range(2, NBLK):
                    xs = xs_pre.pop(blk)
                    if blk + 2 < NBLK:
                        xs_pre[blk + 2] = load_x_groups(
                            blk + 2, XL_GROUPS, XQ_GROUPS, xpool, "x")
                    psums = alloc_ps(0, 8, blk)
                    ots = [opool.tile([128, OF], BF16, tag=f"ot{bs}",
                                      name=f"ot{bs}_{blk}_{it}")
                           for bs in range(2)]
                    last = blk == NBLK - 1
                    if not last:
                        gemm(xs, range(OC),
                             lambda bs, oc: psums[bs * OC + oc])
                        for i in range(8):
                            bs, oc = divmod(i, OC)
                            evict(psums[i], bs, oc, ots, i % 2)
                        for bs in range(2):
                            store(blk, bs, ots, nc.gpsimd)
                    else:
                        # per-psum staggered tail: each accumulator's final
                        # k-tiles run back-to-back so its eviction and store
                        # chunk start while other accumulators still compute
                        NTAIL = 4
                        xkl, xkq = xs
                        for ki, k in enumerate(KORDER[:-NTAIL]):
                            first = ki == 0
                            for bs in range(2):
                                if k < K0:
                                    xt, xj = xkl[k]
                                    lhsT = xt[:, xj,
                                              bs * 128:(bs + 1) * 128]
                                    pm = None
                                else:
                                    xt, xj = xkq[k - K0]
                                    lhsT = xt[:, xj, :,
                                              bs * 128:(bs + 1) * 128]
                                    pm = DR
                                for oc in range(OC):
                                    nc.tensor.matmul(
                                        psums[bs * OC + oc], lhsT,
                                        w_rhs(k, oc),
                                        start=first, stop=False,
                                        perf_mode=pm)
                        tail_ks = KORDER[-NTAIL:]
                        chunk_q = [nc.sync, nc.sync, nc.sync, nc.sync]
                        for i in range(8):
                            bs, oc = divmod(i, OC)
                            for k in tail_ks:
                                if k < K0:
                                    xt, xj = xkl[k]
                                    lhsT = xt[:, xj,
                                              bs * 128:(bs + 1) * 128]
                                    pm = None
                                else:
                                    xt, xj = xkq[k - K0]
                                    lhsT = xt[:, xj, :,
                                              bs * 128:(bs + 1) * 128]
                                    pm = DR
                                nc.tensor.matmul(
                                    psums[i], lhsT, w_rhs(k, oc),
                                    start=False, stop=(k == tail_ks[-1]),
                                    perf_mode=pm)
                            evict(psums[i], bs, oc, ots, i % 2)
                            if i % 2 == 1:
                                row = (blk * 2 + bs) * 128
                                c0 = (oc - 1) * NF
                                chunk_q[(i - 1) // 2].dma_start(
                                    out[row:row + 128, c0:c0 + 2 * NF],
                                    ots[bs][:, c0:c0 + 2 * NF])

    nc.compile()
    return nc


def _get_nc():
    if "nc" not in _NC_CACHE:
        _NC_CACHE["nc"] = build_nc()
    return _NC_CACHE["nc"]


# ------------------------------------------------------------- host prep
def _prep_x(xs):
    """xs [B, 4096] f32 (batch shard) -> xl bf16, xq packed fp8 pairs."""
    xt = np.ascontiguousarray(xs.T) * SX           # [4096, B] scaled
    xl = xt[:KL].astype(NP_BF16)
    q = xt[KL:]
    hi = q.astype(NP_E4)
    lo = ((q - hi.astype(np.float32)) * SL).astype(NP_E4)
    pair = np.stack([hi, lo], axis=1)              # [KQ*128, 2, B]
    xq = np.ascontiguousarray(
        pair.reshape(KQ * 128, 2, B // 256, 256).transpose(0, 2, 1, 3))
    return xl, xq


def _prep_w(ws):
    """ws [OF, 4096] f32 (masked weight shard) -> wl bf16, wq fp8 pairs."""
    wt = np.ascontiguousarray(ws.T) * SW           # [4096, OF] scaled
    wl = wt[:KL].astype(NP_BF16)
    q = wt[KL:]
    hi = q.astype(NP_E4)
    lo = (q / SL).astype(NP_E4)
    # [KQ*128, 2(oc-half), 2(hi/lo), OH] so per-half DMAs are contiguous
    wq = np.ascontiguousarray(
        np.stack([hi.reshape(-1, 2, OH), lo.reshape(-1, 2, OH)], axis=2))
    return wl, wq


def shard_inputs(input, weight, mask):
    x = np.asarray(input, dtype=np.float32)
    s = np.asarray(weight, dtype=np.float32) * np.asarray(mask,
                                                          dtype=np.float32)
    xparts = [_prep_x(x[i * B:(i + 1) * B]) for i in range(B_S)]
    wparts = [_prep_w(s[j * OF:(j + 1) * OF]) for j in range(O_S)]
    in_maps = []
    for c in range(N_CORES):
        xl, xq = xparts[c // O_S]
        wl, wq = wparts[c % O_S]
        in_maps.append({"xl": xl, "xq": xq, "wl": wl, "wq": wq})
    return in_maps


def gather_output(results):
    outp = np.empty((BATCH, OUT_F), np.float32)
    for c in range(N_CORES):
        b0 = (c // O_S) * B
        o0 = (c % O_S) * OF
        outp[b0:b0 + B, o0:o0 + OF] = results[c]["out"].astype(np.float32)
    return outp


def kernel(input, weight, mask):
    from concourse.bass_utils import run_bass_kernel_spmd
    in_maps = shard_inputs(input, weight, mask)
    res = run_bass_kernel_spmd(_get_nc(), in_maps,
                               core_ids=list(range(N_CORES)))
    return gather_output(res.results)


# revision 16
# speedup vs baseline: 1470.1409x; 1.0281x over previous
"""BibdLinear Trainium2 kernel: out = input @ (weight * mask).T

Shapes (hardcoded): input [8192, 4096] f32, weight [4096, 4096] f32,
mask [4096, 4096] f32 -> out [8192, 4096] f32.

Sharding (column-parallel x batch-parallel, 8 cores): 2 batch shards x
4 output-feature shards. Core c handles batch rows [(c//4)*4096, +4096)
and output features [(c%4)*1024, +1024); the host concatenates the 8
output slices.

Per-core device GEMM (Bass/Tile), K=4096 contraction split by dtype:
  - k-tiles 0..16 (K0=17): bf16 operands (x*SX, w*SW planes).
  - k-tiles 17..31 (KQ=15): fp8e4 DoubleRow "hi/lo pair" matmuls:
      stationary pair (x_hi, x_lo*SL), moving pair (w~, w~/SL)
      => psum += x_hi*w~ + x_lo*w~, i.e. x at ~17-bit effective
      precision and only w's single e4m3 quantization (~2.4% rms)
      touching the fp8 fraction. DoubleRow streams at 2x the bf16
      matmul rate, so these k-tiles cost half.
  End-to-end rel err 1.818e-2 vs the f32 reference (gate 2e-2).
  All planes carry a uniform SX*SW scale, descaled at PSUM eviction
  (DVE tensor_scalar_mul / ACT activation-Copy with scale).

Schedule per core: batch blocks of 256 rows; per block 32 k-tiles x
(2 batch subtiles x 4 feature chunks of N=256) accumulate into 8 PSUM
banks. Each accumulator is evicted immediately after its final matmul
(DVE/ACT alternating, staggered so the next block's matmuls never wait
on a bank); a bf16 k-tile is ordered last to widen the eviction window.
Blocks 0-1 are special: they run k-interleaved as two oc-half phases
(both blocks' lo halves, then both hi halves), so the DMA pool only
has to deliver half the weights, spread across a full 21us phase,
while the first blocks compute - removing the weight-preload startup
stall. Weights are
resident in SBUF as per-oc-half k-group tiles; x arrives as per-block
k-group strips with 512B-contiguous descriptors, size-graded (small
first so the first matmul issues ~2us in, large after to respect the
~630ns/DMA HWDGE op rate). Outputs store as bf16 (SWDGE queue), host
upcasts to f32. The last block runs its final 4 k-tiles per-accumulator
(back-to-back per PSUM) so evictions and the two per-subtile stores
start while other accumulators still compute, shrinking the drain
tail. Steady-state x strips prefetch one block ahead, and blocks 2+'s
strips are issued only after the first oc-half phase so they cannot
steal DMA-pool bandwidth from the startup-critical weight stream.

TimelineSim: 355.4us/core (previous fp32r dense baseline: 497.5us).
"""
import numpy as np
import ml_dtypes

import concourse.mybir as mybir
import concourse.tile as tile
from concourse import bacc

# ---------------------------------------------------------------- problem
BATCH, IN_F, OUT_F = 8192, 4096, 4096
B_S, O_S = 2, 4
B, OF = BATCH // B_S, OUT_F // O_S     # 4096, 1024 per core
N_CORES = 8

K0 = 17                                 # bf16 k-tiles
KQ = 32 - K0                            # fp8 DoubleRow k-tiles
KL = K0 * 128
SX, SW, SL = 16.0, 64.0, 32.0

NF = 256                                # matmul moving width
OH = OF // 2                            # oc-half width (512)
F32 = mybir.dt.float32
BF16 = mybir.dt.bfloat16
FP8 = mybir.dt.float8e4
NP_BF16 = ml_dtypes.bfloat16
NP_E4 = ml_dtypes.float8_e4m3

WL_GROUPS = [1, 2, 3, 5, 5, 1]         # bf16 w k-groups (sum K0)
WQ_GROUPS = [4, 5, 6]                  # fp8 w k-groups (sum KQ)
XL_GROUPS0 = [2, 3, 4, 8]              # startup-block bf16 x split
XQ_GROUPS0 = [4, 5, 6]                 # startup-block fp8 x split
XL_GROUPS = [17]                       # steady bf16 x
XQ_GROUPS = [15]                       # steady fp8 x
WARMUP_MMS = 0                         # junk matmuls to ramp the PE p-state

_NC_CACHE = {}


# ---------------------------------------------------------- device program
def build_nc(iters=1, x_bufs=3, out_bufs=4):
    K = IN_F
    KO = K // 128                      # 32
    OC = OF // NF                      # 4
    NBLK = B // 256                    # 16
    SCL = 1.0 / (SX * SW)

    nc = bacc.Bacc(None, target_bir_lowering=False)

    xl = nc.dram_tensor("xl", [KL, B], BF16, kind="ExternalInput")
    xq = nc.dram_tensor("xq", [KQ * 128, B // 256, 2, 256], FP8,
                        kind="ExternalInput")
    wl = nc.dram_tensor("wl", [KL, OF], BF16, kind="ExternalInput")
    wq = nc.dram_tensor("wq", [KQ * 128, 2, 2, OH], FP8,
                        kind="ExternalInput")
    out = nc.dram_tensor("out", [B, OF], BF16, kind="ExternalOutput")

    xlPK = xl.rearrange("(ko p) b -> p ko b", p=128)
    xqPK = xq.rearrange("(kq p) c t b -> p kq c t b", p=128)
    wlPK = wl.rearrange("(ko p) o -> p ko o", p=128)
    wqPK = wq.rearrange("(kq p) h t o -> p kq h t o", p=128)

    DR = mybir.MatmulPerfMode.DoubleRow

    with tile.TileContext(nc) as tc:
        with (
            tc.tile_pool(name="wpool", bufs=1) as wpool,
            tc.tile_pool(name="xpool", bufs=x_bufs) as xpool,
            tc.tile_pool(name="x0pool", bufs=1) as x0pool,
            tc.tile_pool(name="opool", bufs=out_bufs) as opool,
            tc.tile_pool(name="psum", bufs=1, space="PSUM") as psum_pool,
        ):
            for it in range(iters):
                # w k-tile handles per oc-half: wkl[h][k], wkq[h][kq]
                wkl = [[None] * K0 for _ in range(2)]
                wkq = [[None] * KQ for _ in range(2)]

                def load_wl_group(k0, sz, h):
                    wt = wpool.tile([128, sz, OH], BF16, tag=f"wl{k0}h{h}",
                                    name=f"wl{k0}h{h}_{it}")
                    nc.scalar.dma_start(
                        wt, wlPK[:, k0:k0 + sz, h * OH:(h + 1) * OH])
                    for j in range(sz):
                        wkl[h][k0 + j] = (wt, j)

                def load_wq_group(k0, sz, h):
                    wt = wpool.tile([128, sz, 2, OH], FP8, tag=f"wq{k0}h{h}",
                                    name=f"wq{k0}h{h}_{it}")
                    nc.scalar.dma_start(wt, wqPK[:, k0:k0 + sz, h, :, :])
                    for j in range(sz):
                        wkq[h][k0 + j] = (wt, j)

                def load_w_half(h):
                    for gi, sz in enumerate(WL_GROUPS[:-1]):
                        load_wl_group(sum(WL_GROUPS[:gi]), sz, h)
                    for gi, sz in enumerate(WQ_GROUPS):
                        load_wq_group(sum(WQ_GROUPS[:gi]), sz, h)
                    # group holding the last-visited k-tile (K0-1) goes last
                    load_wl_group(sum(WL_GROUPS[:-1]), WL_GROUPS[-1], h)

                def load_x_groups(blk, groups_l, groups_q, pool, tp):
                    xkl = [None] * K0
                    xkq = [None] * KQ
                    k0 = 0
                    for gi, sz in enumerate(groups_l):
                        xt = pool.tile([128, sz, 256], BF16, tag=f"{tp}l{gi}",
                                       name=f"{tp}l{gi}_{blk}_{it}")
                        nc.sync.dma_start(
                            xt, xlPK[:, k0:k0 + sz,
                                     blk * 256:(blk + 1) * 256])
                        for j in range(sz):
                            xkl[k0 + j] = (xt, j)
                        k0 += sz
                    k0 = 0
                    for gi, sz in enumerate(groups_q):
                        xt = pool.tile([128, sz, 2, 256], FP8,
                                       tag=f"{tp}q{gi}",
                                       name=f"{tp}q{gi}_{blk}_{it}")
                        nc.sync.dma_start(xt, xqPK[:, k0:k0 + sz, blk, :, :])
                        for j in range(sz):
                            xkq[k0 + j] = (xt, j)
                        k0 += sz
                    return xkl, xkq

                def w_rhs(k, oc):
                    h, ocl = divmod(oc, OC // 2)
                    if k < K0:
                        wt, j = wkl[h][k]
                        return wt[:, j, ocl * NF:(ocl + 1) * NF]
                    wt, j = wkq[h][k - K0]
                    return wt[:, j, :, ocl * NF:(ocl + 1) * NF]

                KORDER = (list(range(K0 - 1)) + list(range(K0, KO))
                          + [K0 - 1])

                def gemm(xs, ocs, psget):
                    xkl, xkq = xs
                    for ki, k in enumerate(KORDER):
                        first, lastk = ki == 0, ki == KO - 1
                        for bs in range(2):
                            if k < K0:
                                xt, xj = xkl[k]
                                lhsT = xt[:, xj, bs * 128:(bs + 1) * 128]
                                pm = None
                            else:
                                xt, xj = xkq[k - K0]
                                lhsT = xt[:, xj, :, bs * 128:(bs + 1) * 128]
                                pm = DR
                            for oc in ocs:
                                nc.tensor.matmul(
                                    psget(bs, oc), lhsT, w_rhs(k, oc),
                                    start=first, stop=lastk, perf_mode=pm)

                def evict(ps, bs, oc, ots, use_act):
                    dst = ots[bs][:, oc * NF:(oc + 1) * NF]
                    if use_act:
                        nc.scalar.activation(
                            dst, ps, mybir.ActivationFunctionType.Copy,
                            scale=SCL)
                    else:
                        nc.vector.tensor_scalar_mul(dst, ps, SCL)

                def alloc_ps(base, n, blk):
                    return [psum_pool.tile([128, NF], F32, tag=f"ps{base+i}",
                                           name=f"ps{base+i}_{blk}_{it}")
                            for i in range(n)]

                def store(blk, bs, ots, q):
                    q.dma_start(
                        out[(blk * 2 + bs) * 128:(blk * 2 + bs + 1) * 128,
                            :], ots[bs])

                # ---- PE p-state warmup: junk matmuls with no DMA deps ----
                if WARMUP_MMS and it == 0:
                    wm = x0pool.tile([128, 256], BF16, tag="warm",
                                     name=f"warm_{it}")
                    nc.vector.memset(wm[:], 0.0)
                    wps = psum_pool.tile([128, NF], F32, tag="ps7",
                                         name=f"warmps_{it}")
                    for i in range(WARMUP_MMS):
                        nc.tensor.matmul(wps, wm[:, 0:128], wm,
                                         start=(i == 0),
                                         stop=(i == WARMUP_MMS - 1))

                # ---- startup: blocks 0,1 interleaved, one oc-half per
                # phase: halves the early weight demand AND spreads each
                # w-half over a full 2-block phase (21.4us) ------------
                xs_start = [
                    load_x_groups(blk, XL_GROUPS0, XQ_GROUPS0, x0pool,
                                  f"x{'ab'[blk]}")
                    for blk in range(2)
                ]
                load_w_half(0)
                load_w_half(1)
                xs_pre = {}

                ots_start = [
                    [opool.tile([128, OF], BF16, tag=f"ot{bs}",
                                name=f"ot{bs}_{blk}_{it}")
                     for bs in range(2)]
                    for blk in range(2)
                ]
                EVORD = [0, 1, 4, 5, 2, 3, 6, 7]   # psum completion order
                for h in (0, 1):
                    if h == 1:
                        # block2's strip: issued only now so its transfer
                        # stays out of the oh0 phase's DMA window
                        xs_pre[2] = load_x_groups(2, XL_GROUPS, XQ_GROUPS,
                                                  xpool, "x")
                    ocs = [2 * h, 2 * h + 1]
                    ps = alloc_ps(0, 8, 100 + h)
                    for ki, k in enumerate(KORDER):
                        first, lastk = ki == 0, ki == KO - 1
                        for bs in range(2):
                            for blki in range(2):
                                xkl, xkq = xs_start[blki]
                                if k < K0:
                                    xt, xj = xkl[k]
                                    lhsT = xt[:, xj,
                                              bs * 128:(bs + 1) * 128]
                                    pm = None
                                else:
                                    xt, xj = xkq[k - K0]
                                    lhsT = xt[:, xj, :,
                                              bs * 128:(bs + 1) * 128]
                                    pm = DR
                                for oc in ocs:
                                    nc.tensor.matmul(
                                        ps[blki * 4 + bs * 2 + (oc - 2 * h)],
                                        lhsT, w_rhs(k, oc),
                                        start=first, stop=lastk,
                                        perf_mode=pm)
                    for n, i in enumerate(EVORD):
                        blki, r = divmod(i, 4)
                        bs, ocl = divmod(r, 2)
                        evict(ps[i], bs, 2 * h + ocl, ots_start[blki], n % 2)
                    if h == 1:
                        for blk in range(2):
                            for bs in range(2):
                                store(blk, bs, ots_start[blk], nc.gpsimd)

                # ---- steady blocks ---------------------------------------
                for blk in range(2, NBLK):
                    xs = xs_pre.pop(blk)
                    if blk + 1 < NBLK:
                        xs_pre[blk + 1] = load_x_groups(
                            blk + 1, XL_GROUPS, XQ_GROUPS, xpool, "x")
                    psums = alloc_ps(0, 8, blk)
                    ots = [opool.tile([128, OF], BF16, tag=f"ot{bs}",
                                      name=f"ot{bs}_{blk}_{it}")
                           for bs in range(2)]
                    last = blk == NBLK - 1
                    if not last:
                        gemm(xs, range(OC),
                             lambda bs, oc: psums[bs * OC + oc])
                        for i in range(8):
                            bs, oc = divmod(i, OC)
                            evict(psums[i], bs, oc, ots, i % 2)
                        for bs in range(2):
                            store(blk, bs, ots, nc.gpsimd)
                    else:
                        # per-psum staggered tail: each accumulator's final
                        # k-tiles run back-to-back so its eviction and store
                        # chunk start while other accumulators still compute
                        NTAIL = 4
                        xkl, xkq = xs
                        for ki, k in enumerate(KORDER[:-NTAIL]):
                            first = ki == 0
                            for bs in range(2):
                                if k < K0:
                                    xt, xj = xkl[k]
                                    lhsT = xt[:, xj,
                                              bs * 128:(bs + 1) * 128]
                                    pm = None
                                else:
                                    xt, xj = xkq[k - K0]
                                    lhsT = xt[:, xj, :,
                                              bs * 128:(bs + 1) * 128]
                                    pm = DR
                                for oc in range(OC):
                                    nc.tensor.matmul(
                                        psums[bs * OC + oc], lhsT,
                                        w_rhs(k, oc),
                                        start=first, stop=False,
                                        perf_mode=pm)
                        tail_ks = KORDER[-NTAIL:]
                        for i in range(8):
                            bs, oc = divmod(i, OC)
                            for k in tail_ks:
                                if k < K0:
                                    xt, xj = xkl[k]
                                    lhsT = xt[:, xj,
                                              bs * 128:(bs + 1) * 128]
                                    pm = None
                                else:
                                    xt, xj = xkq[k - K0]
                                    lhsT = xt[:, xj, :,
                                              bs * 128:(bs + 1) * 128]
                                    pm = DR
                                nc.tensor.matmul(
                                    psums[i], lhsT, w_rhs(k, oc),
                                    start=False, stop=(k == tail_ks[-1]),
                                    perf_mode=pm)
                            evict(psums[i], bs, oc, ots, i % 2)
                            if i % 4 == 3:
                                store(blk, bs, ots, nc.sync)

    nc.compile()
    return nc


def _get_nc():
    if "nc" not in _NC_CACHE:
        _NC_CACHE["nc"] = build_nc()
    return _NC_CACHE["nc"]


# ------------------------------------------------------------- host prep
def _prep_x(xs):
    """xs [B, 4096] f32 (batch shard) -> xl bf16, xq packed fp8 pairs."""
    xt = np.ascontiguousarray(xs.T) * SX           # [4096, B] scaled
    xl = xt[:KL].astype(NP_BF16)
    q = xt[KL:]
    hi = q.astype(NP_E4)
    lo = ((q - hi.astype(np.float32)) * SL).astype(NP_E4)
    pair = np.stack([hi, lo], axis=1)              # [KQ*128, 2, B]
    xq = np.ascontiguousarray(
        pair.reshape(KQ * 128, 2, B // 256, 256).transpose(0, 2, 1, 3))
    return xl, xq


def _prep_w(ws):
    """ws [OF, 4096] f32 (masked weight shard) -> wl bf16, wq fp8 pairs."""
    wt = np.ascontiguousarray(ws.T) * SW           # [4096, OF] scaled
    wl = wt[:KL].astype(NP_BF16)
    q = wt[KL:]
    hi = q.astype(NP_E4)
    lo = (q / SL).astype(NP_E4)
    # [KQ*128, 2(oc-half), 2(hi/lo), OH] so per-half DMAs are contiguous
    wq = np.ascontiguousarray(
        np.stack([hi.reshape(-1, 2, OH), lo.reshape(-1, 2, OH)], axis=2))
    return wl, wq


def shard_inputs(input, weight, mask):
    x = np.asarray(input, dtype=np.float32)
    s = np.asarray(weight, dtype=np.float32) * np.asarray(mask,
                                                          dtype=np.float32)
    xparts = [_prep_x(x[i * B:(i + 1) * B]) for i in range(B_S)]
    wparts = [_prep_w(s[j * OF:(j + 1) * OF]) for j in range(O_S)]
    in_maps = []
    for c in range(N_CORES):
        xl, xq = xparts[c // O_S]
        wl, wq = wparts[c % O_S]
        in_maps.append({"xl": xl, "xq": xq, "wl": wl, "wq": wq})
    return in_maps


def gather_output(results):
    outp = np.empty((BATCH, OUT_F), np.float32)
    for c in range(N_CORES):
        b0 = (c // O_S) * B
        o0 = (c % O_S) * OF
        outp[b0:b0 + B, o0:o0 + OF] = results[c]["out"].astype(np.float32)
    return outp


def kernel(input, weight, mask):
    from concourse.bass_utils import run_bass_kernel_spmd
    in_maps = shard_inputs(input, weight, mask)
    res = run_bass_kernel_spmd(_get_nc(), in_maps,
                               core_ids=list(range(N_CORES)))
    return gather_output(res.results)


# revision 17
# speedup vs baseline: 1505.0611x; 1.0238x over previous
"""BibdLinear Trainium2 kernel: out = input @ (weight * mask).T

Shapes (hardcoded): input [8192, 4096] f32, weight [4096, 4096] f32,
mask [4096, 4096] f32 -> out [8192, 4096] f32.

Sharding (column-parallel x batch-parallel, 8 cores): 2 batch shards x
4 output-feature shards. Core c handles batch rows [(c//4)*4096, +4096)
and output features [(c%4)*1024, +1024); the host concatenates the 8
output slices.

Per-core device GEMM (Bass/Tile), K=4096 contraction split by dtype:
  - k-tiles 0..15 (K0=16): bf16 operands (x*SX, w*SW planes).
  - k-tiles 16..31 (KQ=16): fp8e4 DoubleRow "hi/lo pair" matmuls:
      stationary pair (x_hi, x_lo*SL), moving pair (w~, w~/SL)
      => psum += x_hi*w~ + x_lo*w~, i.e. x at ~17-bit effective
      precision and only w's single e4m3 quantization (~2.4% rms)
      touching the fp8 fraction. DoubleRow streams at 2x the bf16
      matmul rate, so these k-tiles cost half.
  End-to-end rel err 1.874e-2 vs the f32 reference (gate 2e-2).
  All planes carry a uniform SX*SW scale, descaled at PSUM eviction
  (DVE tensor_scalar_mul / ACT activation-Copy with scale).

Schedule per core: batch blocks of 256 rows; per block 32 k-tiles x
(2 batch subtiles x 4 feature chunks of N=256) accumulate into 8 PSUM
banks. Each accumulator is evicted immediately after its final matmul
(DVE/ACT alternating, staggered so the next block's matmuls never wait
on a bank); a bf16 k-tile is ordered last to widen the eviction window.
Blocks 0-1 are special: they run k-interleaved as two oc-half phases
(both blocks' lo halves, then both hi halves), so the DMA pool only
has to deliver half the weights, spread across a full 21us phase,
while the first blocks compute - removing the weight-preload startup
stall. Weights are
resident in SBUF as per-oc-half k-group tiles; x arrives as per-block
k-group strips with 512B-contiguous descriptors, size-graded (small
first so the first matmul issues ~2us in, large after to respect the
~630ns/DMA HWDGE op rate). Outputs store as bf16 (SWDGE queue), host
upcasts to f32. The last block runs its final 4 k-tiles per-accumulator
(back-to-back per PSUM) so evictions and the two per-subtile stores
start while other accumulators still compute, shrinking the drain
tail. Steady-state x strips prefetch one block ahead, and blocks 2+'s
strips are issued only after the first oc-half phase so they cannot
steal DMA-pool bandwidth from the startup-critical weight stream.

TimelineSim: 347.1us/core (previous fp32r dense baseline: 497.5us).
"""
import numpy as np
import ml_dtypes

import concourse.mybir as mybir
import concourse.tile as tile
from concourse import bacc

# ---------------------------------------------------------------- problem
BATCH, IN_F, OUT_F = 8192, 4096, 4096
B_S, O_S = 2, 4
B, OF = BATCH // B_S, OUT_F // O_S     # 4096, 1024 per core
N_CORES = 8

K0 = 16                                 # bf16 k-tiles
KQ = 32 - K0                            # fp8 DoubleRow k-tiles
KL = K0 * 128
SX, SW, SL = 16.0, 64.0, 32.0

NF = 256                                # matmul moving width
OH = OF // 2                            # oc-half width (512)
F32 = mybir.dt.float32
BF16 = mybir.dt.bfloat16
FP8 = mybir.dt.float8e4
NP_BF16 = ml_dtypes.bfloat16
NP_E4 = ml_dtypes.float8_e4m3

WL_GROUPS = [1, 2, 3, 4, 5, 1]         # bf16 w k-groups (sum K0)
WQ_GROUPS = [4, 5, 7]                  # fp8 w k-groups (sum KQ)
XL_GROUPS0 = [2, 3, 4, 7]              # startup-block bf16 x split
XQ_GROUPS0 = [4, 5, 7]                 # startup-block fp8 x split
XL_GROUPS = [16]                       # steady bf16 x
XQ_GROUPS = [16]                       # steady fp8 x
WARMUP_MMS = 0                         # junk matmuls to ramp the PE p-state

_NC_CACHE = {}


# ---------------------------------------------------------- device program
def build_nc(iters=1, x_bufs=3, out_bufs=4):
    K = IN_F
    KO = K // 128                      # 32
    OC = OF // NF                      # 4
    NBLK = B // 256                    # 16
    SCL = 1.0 / (SX * SW)

    nc = bacc.Bacc(None, target_bir_lowering=False)

    xl = nc.dram_tensor("xl", [KL, B], BF16, kind="ExternalInput")
    xq = nc.dram_tensor("xq", [KQ * 128, B // 256, 2, 256], FP8,
                        kind="ExternalInput")
    wl = nc.dram_tensor("wl", [KL, OF], BF16, kind="ExternalInput")
    wq = nc.dram_tensor("wq", [KQ * 128, 2, 2, OH], FP8,
                        kind="ExternalInput")
    out = nc.dram_tensor("out", [B, OF], BF16, kind="ExternalOutput")

    xlPK = xl.rearrange("(ko p) b -> p ko b", p=128)
    xqPK = xq.rearrange("(kq p) c t b -> p kq c t b", p=128)
    wlPK = wl.rearrange("(ko p) o -> p ko o", p=128)
    wqPK = wq.rearrange("(kq p) h t o -> p kq h t o", p=128)

    DR = mybir.MatmulPerfMode.DoubleRow

    with tile.TileContext(nc) as tc:
        with (
            tc.tile_pool(name="wpool", bufs=1) as wpool,
            tc.tile_pool(name="xpool", bufs=x_bufs) as xpool,
            tc.tile_pool(name="x0pool", bufs=1) as x0pool,
            tc.tile_pool(name="opool", bufs=out_bufs) as opool,
            tc.tile_pool(name="psum", bufs=1, space="PSUM") as psum_pool,
        ):
            for it in range(iters):
                # w k-tile handles per oc-half: wkl[h][k], wkq[h][kq]
                wkl = [[None] * K0 for _ in range(2)]
                wkq = [[None] * KQ for _ in range(2)]

                def load_wl_group(k0, sz, h):
                    wt = wpool.tile([128, sz, OH], BF16, tag=f"wl{k0}h{h}",
                                    name=f"wl{k0}h{h}_{it}")
                    nc.scalar.dma_start(
                        wt, wlPK[:, k0:k0 + sz, h * OH:(h + 1) * OH])
                    for j in range(sz):
                        wkl[h][k0 + j] = (wt, j)

                def load_wq_group(k0, sz, h):
                    wt = wpool.tile([128, sz, 2, OH], FP8, tag=f"wq{k0}h{h}",
                                    name=f"wq{k0}h{h}_{it}")
                    nc.scalar.dma_start(wt, wqPK[:, k0:k0 + sz, h, :, :])
                    for j in range(sz):
                        wkq[h][k0 + j] = (wt, j)

                def load_w_half(h):
                    for gi, sz in enumerate(WL_GROUPS[:-1]):
                        load_wl_group(sum(WL_GROUPS[:gi]), sz, h)
                    for gi, sz in enumerate(WQ_GROUPS):
                        load_wq_group(sum(WQ_GROUPS[:gi]), sz, h)
                    # group holding the last-visited k-tile (K0-1) goes last
                    load_wl_group(sum(WL_GROUPS[:-1]), WL_GROUPS[-1], h)

                def load_x_groups(blk, groups_l, groups_q, pool, tp):
                    xkl = [None] * K0
                    xkq = [None] * KQ
                    k0 = 0
                    for gi, sz in enumerate(groups_l):
                        xt = pool.tile([128, sz, 256], BF16, tag=f"{tp}l{gi}",
                                       name=f"{tp}l{gi}_{blk}_{it}")
                        nc.sync.dma_start(
                            xt, xlPK[:, k0:k0 + sz,
                                     blk * 256:(blk + 1) * 256])
                        for j in range(sz):
                            xkl[k0 + j] = (xt, j)
                        k0 += sz
                    k0 = 0
                    for gi, sz in enumerate(groups_q):
                        xt = pool.tile([128, sz, 2, 256], FP8,
                                       tag=f"{tp}q{gi}",
                                       name=f"{tp}q{gi}_{blk}_{it}")
                        nc.sync.dma_start(xt, xqPK[:, k0:k0 + sz, blk, :, :])
                        for j in range(sz):
                            xkq[k0 + j] = (xt, j)
                        k0 += sz
                    return xkl, xkq

                def w_rhs(k, oc):
                    h, ocl = divmod(oc, OC // 2)
                    if k < K0:
                        wt, j = wkl[h][k]
                        return wt[:, j, ocl * NF:(ocl + 1) * NF]
                    wt, j = wkq[h][k - K0]
                    return wt[:, j, :, ocl * NF:(ocl + 1) * NF]

                KORDER = (list(range(K0 - 1)) + list(range(K0, KO))
                          + [K0 - 1])

                def gemm(xs, ocs, psget):
                    xkl, xkq = xs
                    for ki, k in enumerate(KORDER):
                        first, lastk = ki == 0, ki == KO - 1
                        for bs in range(2):
                            if k < K0:
                                xt, xj = xkl[k]
                                lhsT = xt[:, xj, bs * 128:(bs + 1) * 128]
                                pm = None
                            else:
                                xt, xj = xkq[k - K0]
                                lhsT = xt[:, xj, :, bs * 128:(bs + 1) * 128]
                                pm = DR
                            for oc in ocs:
                                nc.tensor.matmul(
                                    psget(bs, oc), lhsT, w_rhs(k, oc),
                                    start=first, stop=lastk, perf_mode=pm)

                def evict(ps, bs, oc, ots, use_act):
                    dst = ots[bs][:, oc * NF:(oc + 1) * NF]
                    if use_act:
                        nc.scalar.activation(
                            dst, ps, mybir.ActivationFunctionType.Copy,
                            scale=SCL)
                    else:
                        nc.vector.tensor_scalar_mul(dst, ps, SCL)

                def alloc_ps(base, n, blk):
                    return [psum_pool.tile([128, NF], F32, tag=f"ps{base+i}",
                                           name=f"ps{base+i}_{blk}_{it}")
                            for i in range(n)]

                def store(blk, bs, ots, q):
                    q.dma_start(
                        out[(blk * 2 + bs) * 128:(blk * 2 + bs + 1) * 128,
                            :], ots[bs])

                # ---- PE p-state warmup: junk matmuls with no DMA deps ----
                if WARMUP_MMS and it == 0:
                    wm = x0pool.tile([128, 256], BF16, tag="warm",
                                     name=f"warm_{it}")
                    nc.vector.memset(wm[:], 0.0)
                    wps = psum_pool.tile([128, NF], F32, tag="ps7",
                                         name=f"warmps_{it}")
                    for i in range(WARMUP_MMS):
                        nc.tensor.matmul(wps, wm[:, 0:128], wm,
                                         start=(i == 0),
                                         stop=(i == WARMUP_MMS - 1))

                # ---- startup: blocks 0,1 interleaved, one oc-half per
                # phase: halves the early weight demand AND spreads each
                # w-half over a full 2-block phase (21.4us) ------------
                xs_start = [
                    load_x_groups(blk, XL_GROUPS0, XQ_GROUPS0, x0pool,
                                  f"x{'ab'[blk]}")
                    for blk in range(2)
                ]
                load_w_half(0)
                load_w_half(1)
                xs_pre = {}

                ots_start = [
                    [opool.tile([128, OF], BF16, tag=f"ot{bs}",
                                name=f"ot{bs}_{blk}_{it}")
                     for bs in range(2)]
                    for blk in range(2)
                ]
                EVORD = [0, 1, 4, 5, 2, 3, 6, 7]   # psum completion order
                for h in (0, 1):
                    if h == 1:
                        # block2's strip: issued only now so its transfer
                        # stays out of the oh0 phase's DMA window
                        xs_pre[2] = load_x_groups(2, XL_GROUPS, XQ_GROUPS,
                                                  xpool, "x")
                    ocs = [2 * h, 2 * h + 1]
                    ps = alloc_ps(0, 8, 100 + h)
                    for ki, k in enumerate(KORDER):
                        first, lastk = ki == 0, ki == KO - 1
                        for bs in range(2):
                            for blki in range(2):
                                xkl, xkq = xs_start[blki]
                                if k < K0:
                                    xt, xj = xkl[k]
                                    lhsT = xt[:, xj,
                                              bs * 128:(bs + 1) * 128]
                                    pm = None
                                else:
                                    xt, xj = xkq[k - K0]
                                    lhsT = xt[:, xj, :,
                                              bs * 128:(bs + 1) * 128]
                                    pm = DR
                                for oc in ocs:
                                    nc.tensor.matmul(
                                        ps[blki * 4 + bs * 2 + (oc - 2 * h)],
                                        lhsT, w_rhs(k, oc),
                                        start=first, stop=lastk,
                                        perf_mode=pm)
                    for n, i in enumerate(EVORD):
                        blki, r = divmod(i, 4)
                        bs, ocl = divmod(r, 2)
                        evict(ps[i], bs, 2 * h + ocl, ots_start[blki], n % 2)
                    if h == 1:
                        for blk in range(2):
                            for bs in range(2):
                                store(blk, bs, ots_start[blk], nc.gpsimd)

                # ---- steady blocks ---------------------------------------
                for blk in range(2, NBLK):
                    xs = xs_pre.pop(blk)
                    if blk + 1 < NBLK:
                        xs_pre[blk + 1] = load_x_groups(
                            blk + 1, XL_GROUPS, XQ_GROUPS, xpool, "x")
                    psums = alloc_ps(0, 8, blk)
                    ots = [opool.tile([128, OF], BF16, tag=f"ot{bs}",
                                      name=f"ot{bs}_{blk}_{it}")
                           for bs in range(2)]
                    last = blk == NBLK - 1
                    if not last:
                        gemm(xs, range(OC),
                             lambda bs, oc: psums[bs * OC + oc])
                        for i in range(8):
                            bs, oc = divmod(i, OC)
                            evict(psums[i], bs, oc, ots, i % 2)
                        for bs in range(2):
                            store(blk, bs, ots, nc.gpsimd)
                    else:
                        # per-psum staggered tail: each accumulator's final
                        # k-tiles run back-to-back so its eviction and store
                        # chunk start while other accumulators still compute
                        NTAIL = 4
                        xkl, xkq = xs
                        for ki, k in enumerate(KORDER[:-NTAIL]):
                            first = ki == 0
                            for bs in range(2):
                                if k < K0:
                                    xt, xj = xkl[k]
                                    lhsT = xt[:, xj,
                                              bs * 128:(bs + 1) * 128]
                                    pm = None
                                else:
                                    xt, xj = xkq[k - K0]
                                    lhsT = xt[:, xj, :,
                                              bs * 128:(bs + 1) * 128]
                                    pm = DR
                                for oc in range(OC):
                                    nc.tensor.matmul(
                                        psums[bs * OC + oc], lhsT,
                                        w_rhs(k, oc),
                                        start=first, stop=False,
                                        perf_mode=pm)
                        tail_ks = KORDER[-NTAIL:]
                        for i in range(8):
                            bs, oc = divmod(i, OC)
                            for k in tail_ks:
                                if k < K0:
                                    xt, xj = xkl[k]
                                    lhsT = xt[:, xj,
                                              bs * 128:(bs + 1) * 128]
                                    pm = None
                                else:
                                    xt, xj = xkq[k - K0]
                                    lhsT = xt[:, xj, :,
                                              bs * 128:(bs + 1) * 128]
                                    pm = DR
                                nc.tensor.matmul(
                                    psums[i], lhsT, w_rhs(k, oc),
                                    start=False, stop=(k == tail_ks[-1]),
                                    perf_mode=pm)
                            evict(psums[i], bs, oc, ots, i % 2)
                            if i % 4 == 3:
                                store(blk, bs, ots, nc.sync)

    nc.compile()
    return nc


def _get_nc():
    if "nc" not in _NC_CACHE:
        _NC_CACHE["nc"] = build_nc()
    return _NC_CACHE["nc"]


# ------------------------------------------------------------- host prep
def _prep_x(xs):
    """xs [B, 4096] f32 (batch shard) -> xl bf16, xq packed fp8 pairs."""
    xt = np.ascontiguousarray(xs.T) * SX           # [4096, B] scaled
    xl = xt[:KL].astype(NP_BF16)
    q = xt[KL:]
    hi = q.astype(NP_E4)
    lo = ((q - hi.astype(np.float32)) * SL).astype(NP_E4)
    pair = np.stack([hi, lo], axis=1)              # [KQ*128, 2, B]
    xq = np.ascontiguousarray(
        pair.reshape(KQ * 128, 2, B // 256, 256).transpose(0, 2, 1, 3))
    return xl, xq


def _prep_w(ws):
    """ws [OF, 4096] f32 (masked weight shard) -> wl bf16, wq fp8 pairs."""
    wt = np.ascontiguousarray(ws.T) * SW           # [4096, OF] scaled
    wl = wt[:KL].astype(NP_BF16)
    q = wt[KL:]
    hi = q.astype(NP_E4)
    lo = (q / SL).astype(NP_E4)
    # [KQ*128, 2(oc-half), 2(hi/lo), OH] so per-half DMAs are contiguous
    wq = np.ascontiguousarray(
        np.stack([hi.reshape(-1, 2, OH), lo.reshape(-1, 2, OH)], axis=2))
    return wl, wq


def shard_inputs(input, weight, mask):
    x = np.asarray(input, dtype=np.float32)
    s = np.asarray(weight, dtype=np.float32) * np.asarray(mask,
                                                          dtype=np.float32)
    xparts = [_prep_x(x[i * B:(i + 1) * B]) for i in range(B_S)]
    wparts = [_prep_w(s[j * OF:(j + 1) * OF]) for j in range(O_S)]
    in_maps = []
    for c in range(N_CORES):
        xl, xq = xparts[c // O_S]
        wl, wq = wparts[c % O_S]
        in_maps.append({"xl": xl, "xq": xq, "wl": wl, "wq": wq})
    return in_maps


def gather_output(results):
    outp = np.empty((BATCH, OUT_F), np.float32)
    for c in range(N_CORES):
        b0 = (c // O_S) * B
        o0 = (c % O_S) * OF
        outp[b0:b0 + B, o0:o0 + OF] = results[c]["out"].astype(np.float32)
    return outp


def kernel(input, weight, mask):
    from concourse.bass_utils import run_bass_kernel_spmd
    in_maps = shard_inputs(input, weight, mask)
    res = run_bass_kernel_spmd(_get_nc(), in_maps,
                               core_ids=list(range(N_CORES)))
    return gather_output(res.results)


# revision 21
# speedup vs baseline: 1505.7378x; 1.0004x over previous
"""BibdLinear Trainium2 kernel: out = input @ (weight * mask).T

Shapes (hardcoded): input [8192, 4096] f32, weight [4096, 4096] f32,
mask [4096, 4096] f32 -> out [8192, 4096] f32.

Sharding (column-parallel x batch-parallel, 8 cores): 2 batch shards x
4 output-feature shards. Core c handles batch rows [(c//4)*4096, +4096)
and output features [(c%4)*1024, +1024); the host concatenates the 8
output slices.

Per-core device GEMM (Bass/Tile), K=4096 contraction split by dtype:
  - k-tiles 0..15 (K0=16): bf16 operands (x*SX, w*SW planes).
  - k-tiles 16..31 (KQ=16): fp8e4 DoubleRow "hi/lo pair" matmuls:
      stationary pair (x_hi, x_lo*SL), moving pair (w~, w~/SL)
      => psum += x_hi*w~ + x_lo*w~, i.e. x at ~17-bit effective
      precision and only w's single e4m3 quantization (~2.4% rms)
      touching the fp8 fraction. DoubleRow streams at 2x the bf16
      matmul rate, so these k-tiles cost half.
  End-to-end rel err 1.874e-2 vs the f32 reference (gate 2e-2).
  All planes carry a uniform SX*SW scale, descaled at PSUM eviction
  (DVE tensor_scalar_mul / ACT activation-Copy with scale).

Schedule per core: batch blocks of 256 rows; per block 32 k-tiles x
(2 batch subtiles x 4 feature chunks of N=256) accumulate into 8 PSUM
banks. Each accumulator is evicted immediately after its final matmul
(DVE/ACT alternating, staggered so the next block's matmuls never wait
on a bank); a bf16 k-tile is ordered last to widen the eviction window.
Blocks 0-1 are special: they run k-interleaved as two oc-half phases
(both blocks' lo halves, then both hi halves), so the DMA pool only
has to deliver half the weights, spread across a full 21us phase,
while the first blocks compute - removing the weight-preload startup
stall. Weights are
resident in SBUF as per-oc-half k-group tiles; x arrives as per-block
k-group strips with 512B-contiguous descriptors, size-graded (small
first so the first matmul issues ~2us in, large after to respect the
~630ns/DMA HWDGE op rate). Outputs store as bf16 (SWDGE queue), host
upcasts to f32. The last block runs its final 4 k-tiles per-accumulator
(back-to-back per PSUM) so evictions and the two per-subtile stores
start while other accumulators still compute, shrinking the drain
tail. Steady-state x strips prefetch one block ahead, and blocks 2+'s
strips are issued only after the first oc-half phase so they cannot
steal DMA-pool bandwidth from the startup-critical weight stream.

TimelineSim: 347.0us/core (previous fp32r dense baseline: 497.5us).
"""
import numpy as np
import ml_dtypes

import concourse.mybir as mybir
import concourse.tile as tile
from concourse import bacc

# ---------------------------------------------------------------- problem
BATCH, IN_F, OUT_F = 8192, 4096, 4096
B_S, O_S = 2, 4
B, OF = BATCH // B_S, OUT_F // O_S     # 4096, 1024 per core
N_CORES = 8

K0 = 16                                 # bf16 k-tiles
KQ = 32 - K0                            # fp8 DoubleRow k-tiles
KL = K0 * 128
SX, SW, SL = 16.0, 64.0, 32.0

NF = 256                                # matmul moving width
OH = OF // 2                            # oc-half width (512)
F32 = mybir.dt.float32
BF16 = mybir.dt.bfloat16
FP8 = mybir.dt.float8e4
NP_BF16 = ml_dtypes.bfloat16
NP_E4 = ml_dtypes.float8_e4m3

WL_GROUPS = [1, 2, 3, 4, 5, 1]         # bf16 w k-groups (sum K0)
WQ_GROUPS = [4, 5, 7]                  # fp8 w k-groups (sum KQ)
XL_GROUPS0 = [2, 3, 4, 7]              # startup-block bf16 x split
XQ_GROUPS0 = [4, 5, 7]                 # startup-block fp8 x split
XL_GROUPS = [16]                       # steady bf16 x
XQ_GROUPS = [16]                       # steady fp8 x
WARMUP_MMS = 0                         # junk matmuls to ramp the PE p-state

_NC_CACHE = {}


# ---------------------------------------------------------- device program
def build_nc(iters=1, x_bufs=3, out_bufs=6):
    K = IN_F
    KO = K // 128                      # 32
    OC = OF // NF                      # 4
    NBLK = B // 256                    # 16
    SCL = 1.0 / (SX * SW)

    nc = bacc.Bacc(None, target_bir_lowering=False)

    xl = nc.dram_tensor("xl", [KL, B], BF16, kind="ExternalInput")
    xq = nc.dram_tensor("xq", [KQ * 128, B // 256, 2, 256], FP8,
                        kind="ExternalInput")
    wl = nc.dram_tensor("wl", [KL, OF], BF16, kind="ExternalInput")
    wq = nc.dram_tensor("wq", [KQ * 128, 2, 2, OH], FP8,
                        kind="ExternalInput")
    out = nc.dram_tensor("out", [B, OF], BF16, kind="ExternalOutput")

    xlPK = xl.rearrange("(ko p) b -> p ko b", p=128)
    xqPK = xq.rearrange("(kq p) c t b -> p kq c t b", p=128)
    wlPK = wl.rearrange("(ko p) o -> p ko o", p=128)
    wqPK = wq.rearrange("(kq p) h t o -> p kq h t o", p=128)

    DR = mybir.MatmulPerfMode.DoubleRow

    with tile.TileContext(nc) as tc:
        with (
            tc.tile_pool(name="wpool", bufs=1) as wpool,
            tc.tile_pool(name="xpool", bufs=x_bufs) as xpool,
            tc.tile_pool(name="x0pool", bufs=1) as x0pool,
            tc.tile_pool(name="opool", bufs=out_bufs) as opool,
            tc.tile_pool(name="psum", bufs=1, space="PSUM") as psum_pool,
        ):
            for it in range(iters):
                # w k-tile handles per oc-half: wkl[h][k], wkq[h][kq]
                wkl = [[None] * K0 for _ in range(2)]
                wkq = [[None] * KQ for _ in range(2)]

                def load_wl_group(k0, sz, h):
                    wt = wpool.tile([128, sz, OH], BF16, tag=f"wl{k0}h{h}",
                                    name=f"wl{k0}h{h}_{it}")
                    nc.scalar.dma_start(
                        wt, wlPK[:, k0:k0 + sz, h * OH:(h + 1) * OH])
                    for j in range(sz):
                        wkl[h][k0 + j] = (wt, j)

                def load_wq_group(k0, sz, h):
                    wt = wpool.tile([128, sz, 2, OH], FP8, tag=f"wq{k0}h{h}",
                                    name=f"wq{k0}h{h}_{it}")
                    nc.scalar.dma_start(wt, wqPK[:, k0:k0 + sz, h, :, :])
                    for j in range(sz):
                        wkq[h][k0 + j] = (wt, j)

                def load_w_half(h):
                    for gi, sz in enumerate(WL_GROUPS[:-1]):
                        load_wl_group(sum(WL_GROUPS[:gi]), sz, h)
                    for gi, sz in enumerate(WQ_GROUPS):
                        load_wq_group(sum(WQ_GROUPS[:gi]), sz, h)
                    # group holding the last-visited k-tile (K0-1) goes last
                    load_wl_group(sum(WL_GROUPS[:-1]), WL_GROUPS[-1], h)

                def load_x_groups(blk, groups_l, groups_q, pool, tp):
                    xkl = [None] * K0
                    xkq = [None] * KQ
                    k0 = 0
                    for gi, sz in enumerate(groups_l):
                        xt = pool.tile([128, sz, 256], BF16, tag=f"{tp}l{gi}",
                                       name=f"{tp}l{gi}_{blk}_{it}")
                        nc.sync.dma_start(
                            xt, xlPK[:, k0:k0 + sz,
                                     blk * 256:(blk + 1) * 256])
                        for j in range(sz):
                            xkl[k0 + j] = (xt, j)
                        k0 += sz
                    k0 = 0
                    for gi, sz in enumerate(groups_q):
                        xt = pool.tile([128, sz, 2, 256], FP8,
                                       tag=f"{tp}q{gi}",
                                       name=f"{tp}q{gi}_{blk}_{it}")
                        nc.sync.dma_start(xt, xqPK[:, k0:k0 + sz, blk, :, :])
                        for j in range(sz):
                            xkq[k0 + j] = (xt, j)
                        k0 += sz
                    return xkl, xkq

                def w_rhs(k, oc):
                    h, ocl = divmod(oc, OC // 2)
                    if k < K0:
                        wt, j = wkl[h][k]
                        return wt[:, j, ocl * NF:(ocl + 1) * NF]
                    wt, j = wkq[h][k - K0]
                    return wt[:, j, :, ocl * NF:(ocl + 1) * NF]

                KORDER = (list(range(K0 - 1)) + list(range(K0, KO))
                          + [K0 - 1])

                def gemm(xs, ocs, psget):
                    xkl, xkq = xs
                    for ki, k in enumerate(KORDER):
                        first, lastk = ki == 0, ki == KO - 1
                        for bs in range(2):
                            if k < K0:
                                xt, xj = xkl[k]
                                lhsT = xt[:, xj, bs * 128:(bs + 1) * 128]
                                pm = None
                            else:
                                xt, xj = xkq[k - K0]
                                lhsT = xt[:, xj, :, bs * 128:(bs + 1) * 128]
                                pm = DR
                            for oc in ocs:
                                nc.tensor.matmul(
                                    psget(bs, oc), lhsT, w_rhs(k, oc),
                                    start=first, stop=lastk, perf_mode=pm)

                def evict(ps, bs, oc, ots, use_act):
                    dst = ots[bs][:, oc * NF:(oc + 1) * NF]
                    if use_act:
                        nc.scalar.activation(
                            dst, ps, mybir.ActivationFunctionType.Copy,
                            scale=SCL)
                    else:
                        nc.vector.tensor_scalar_mul(dst, ps, SCL)

                def alloc_ps(base, n, blk):
                    return [psum_pool.tile([128, NF], F32, tag=f"ps{base+i}",
                                           name=f"ps{base+i}_{blk}_{it}")
                            for i in range(n)]

                def store(blk, bs, ots, q):
                    q.dma_start(
                        out[(blk * 2 + bs) * 128:(blk * 2 + bs + 1) * 128,
                            :], ots[bs])

                # ---- PE p-state warmup: junk matmuls with no DMA deps ----
                if WARMUP_MMS and it == 0:
                    wm = x0pool.tile([128, 256], BF16, tag="warm",
                                     name=f"warm_{it}")
                    nc.vector.memset(wm[:], 0.0)
                    wps = psum_pool.tile([128, NF], F32, tag="ps7",
                                         name=f"warmps_{it}")
                    for i in range(WARMUP_MMS):
                        nc.tensor.matmul(wps, wm[:, 0:128], wm,
                                         start=(i == 0),
                                         stop=(i == WARMUP_MMS - 1))

                # ---- startup: blocks 0,1 interleaved, one oc-half per
                # phase: halves the early weight demand AND spreads each
                # w-half over a full 2-block phase (21.4us) ------------
                xs_start = [
                    load_x_groups(blk, XL_GROUPS0, XQ_GROUPS0, x0pool,
                                  f"x{'ab'[blk]}")
                    for blk in range(2)
                ]
                load_w_half(0)
                load_w_half(1)
                xs_pre = {}

                ots_start = [
                    [opool.tile([128, OF], BF16, tag=f"ot{bs}",
                                name=f"ot{bs}_{blk}_{it}")
                     for bs in range(2)]
                    for blk in range(2)
                ]
                EVORD = [0, 1, 4, 5, 2, 3, 6, 7]   # psum completion order
                for h in (0, 1):
                    if h == 1:
                        # block2's strip: issued only now so its transfer
                        # stays out of the oh0 phase's DMA window
                        xs_pre[2] = load_x_groups(2, XL_GROUPS, XQ_GROUPS,
                                                  xpool, "x")
                    ocs = [2 * h, 2 * h + 1]
                    ps = alloc_ps(0, 8, 100 + h)
                    for ki, k in enumerate(KORDER):
                        first, lastk = ki == 0, ki == KO - 1
                        for bs in range(2):
                            for blki in range(2):
                                xkl, xkq = xs_start[blki]
                                if k < K0:
                                    xt, xj = xkl[k]
                                    lhsT = xt[:, xj,
                                              bs * 128:(bs + 1) * 128]
                                    pm = None
                                else:
                                    xt, xj = xkq[k - K0]
                                    lhsT = xt[:, xj, :,
                                              bs * 128:(bs + 1) * 128]
                                    pm = DR
                                for oc in ocs:
                                    nc.tensor.matmul(
                                        ps[blki * 4 + bs * 2 + (oc - 2 * h)],
                                        lhsT, w_rhs(k, oc),
                                        start=first, stop=lastk,
                                        perf_mode=pm)
                    for n, i in enumerate(EVORD):
                        blki, r = divmod(i, 4)
                        bs, ocl = divmod(r, 2)
                        evict(ps[i], bs, 2 * h + ocl, ots_start[blki], n % 2)
                    if h == 1:
                        for blk in range(2):
                            for bs in range(2):
                                store(blk, bs, ots_start[blk], nc.gpsimd)

                # ---- steady blocks ---------------------------------------
                for blk in range(2, NBLK):
                    xs = xs_pre.pop(blk)
                    if blk + 1 < NBLK:
                        xs_pre[blk + 1] = load_x_groups(
                            blk + 1, XL_GROUPS, XQ_GROUPS, xpool, "x")
                    psums = alloc_ps(0, 8, blk)
                    ots = [opool.tile([128, OF], BF16, tag=f"ot{bs}",
                                      name=f"ot{bs}_{blk}_{it}")
                           for bs in range(2)]
                    last = blk == NBLK - 1
                    if not last:
                        gemm(xs, range(OC),
                             lambda bs, oc: psums[bs * OC + oc])
                        for i in range(8):
                            bs, oc = divmod(i, OC)
                            evict(psums[i], bs, oc, ots, i % 2)
                        for bs in range(2):
                            store(blk, bs, ots, nc.gpsimd)
                    else:
                        # per-psum staggered tail: each accumulator's final
                        # k-tiles run back-to-back so its eviction and store
                        # chunk start while other accumulators still compute
                        NTAIL = 4
                        xkl, xkq = xs
                        for ki, k in enumerate(KORDER[:-NTAIL]):
                            first = ki == 0
                            for bs in range(2):
                                if k < K0:
                                    xt, xj = xkl[k]
                                    lhsT = xt[:, xj,
                                              bs * 128:(bs + 1) * 128]
                                    pm = None
                                else:
                                    xt, xj = xkq[k - K0]
                                    lhsT = xt[:, xj, :,
                                              bs * 128:(bs + 1) * 128]
                                    pm = DR
                                for oc in range(OC):
                                    nc.tensor.matmul(
                                        psums[bs * OC + oc], lhsT,
                                        w_rhs(k, oc),
                                        start=first, stop=False,
                                        perf_mode=pm)
                        tail_ks = KORDER[-NTAIL:]
                        for i in range(8):
                            bs, oc = divmod(i, OC)
                            for k in tail_ks:
                                if k < K0:
                                    xt, xj = xkl[k]
                                    lhsT = xt[:, xj,
                                              bs * 128:(bs + 1) * 128]
                                    pm = None
                                else:
                                    xt, xj = xkq[k - K0]
                                    lhsT = xt[:, xj, :,
                                              bs * 128:(bs + 1) * 128]
                                    pm = DR
                                nc.tensor.matmul(
                                    psums[i], lhsT, w_rhs(k, oc),
                                    start=False, stop=(k == tail_ks[-1]),
                                    perf_mode=pm)
                            evict(psums[i], bs, oc, ots, i % 2)
                            if i % 4 == 3:
                                store(blk, bs, ots, nc.sync)

    nc.compile()
    return nc


def _get_nc():
    if "nc" not in _NC_CACHE:
        _NC_CACHE["nc"] = build_nc()
    return _NC_CACHE["nc"]


# ------------------------------------------------------------- host prep
def _prep_x(xs):
    """xs [B, 4096] f32 (batch shard) -> xl bf16, xq packed fp8 pairs."""
    xt = np.ascontiguousarray(xs.T) * SX           # [4096, B] scaled
    xl = xt[:KL].astype(NP_BF16)
    q = xt[KL:]
    hi = q.astype(NP_E4)
    lo = ((q - hi.astype(np.float32)) * SL).astype(NP_E4)
    pair = np.stack([hi, lo], axis=1)              # [KQ*128, 2, B]
    xq = np.ascontiguousarray(
        pair.reshape(KQ * 128, 2, B // 256, 256).transpose(0, 2, 1, 3))
    return xl, xq


def _prep_w(ws):
    """ws [OF, 4096] f32 (masked weight shard) -> wl bf16, wq fp8 pairs."""
    wt = np.ascontiguousarray(ws.T) * SW           # [4096, OF] scaled
    wl = wt[:KL].astype(NP_BF16)
    q = wt[KL:]
    hi = q.astype(NP_E4)
    lo = (q / SL).astype(NP_E4)
    # [KQ*128, 2(oc-half), 2(hi/lo), OH] so per-half DMAs are contiguous
    wq = np.ascontiguousarray(
        np.stack([hi.reshape(-1, 2, OH), lo.reshape(-1, 2, OH)], axis=2))
    return wl, wq


def shard_inputs(input, weight, mask):
    x = np.asarray(input, dtype=np.float32)
    s = np.asarray(weight, dtype=np.float32) * np.asarray(mask,
                                                          dtype=np.float32)
    xparts = [_prep_x(x[i * B:(i + 1) * B]) for i in range(B_S)]
    wparts = [_prep_w(s[j * OF:(j + 1) * OF]) for j in range(O_S)]
    in_maps = []
    for c in range(N_CORES):
        xl, xq = xparts[c // O_S]
        wl, wq = wparts[c % O_S]
        in_maps.append({"xl": xl, "xq": xq, "wl": wl, "wq": wq})
    return in_maps


def gather_output(results):
    outp = np.empty((BATCH, OUT_F), np.float32)
    for c in range(N_CORES):
        b0 = (c // O_S) * B
        o0 = (c % O_S) * OF
        outp[b0:b0 + B, o0:o0 + OF] = results[c]["out"].astype(np.float32)
    return outp


def kernel(input, weight, mask):
    from concourse.bass_utils import run_bass_kernel_spmd
    in_maps = shard_inputs(input, weight, mask)
    res = run_bass_kernel_spmd(_get_nc(), in_maps,
                               core_ids=list(range(N_CORES)))
    return gather_output(res.results)
